# revision 21
# baseline (speedup 1.0000x reference)
"""Trainium2 Bass kernel for the dual-attention module (spatial + channel attention).

Contract: kernel(**inputs) takes the FULL inputs (x: (16,1024,64,64) f32 plus four
1x1-conv weight matrices) and returns the FULL output (16,1024,64,64) f32.
Internally shards data-parallel over batch across 8 NeuronCores (2 samples/core),
weights replicated.

Per-sample math (b, c=1024, ch=512, hw=4096):
  conv(w) = relu(w @ X)               X = x[b] as (1024, 4096)
  mask    = softmax(conv(w_qr))       over hw          (spatial attn branch)
  ctx     = conv(w_vr) @ mask         (ch,)
  s       = sigmoid(layernorm(ctx))   (ch,)
  avg     = softmax(mean_hw(conv(w_ql)))               (channel attn branch)
  chan    = sigmoid(avg @ conv(w_vl)) (hw,)
  out[0:512]    = x * (1 + s*chan)                     ("sequence")
  out[512:1024] = x * (1 + s + chan)                   ("parallel")

Kernel strategy per core:
  - x sample resident in SBUF as 64 [128,512] f32 tiles (8 k-tiles x 8 n-chunks).
  - All matmuls in fp32r (full PE rate at N=512).
  - Softmaxes are computed unnormalized (exp only); the 1/Z scales are folded
    into later scalar multiplies, so no [1,N] single-partition ops anywhere.
  - qr conv uses a column-replicated weight so its psum rows are broadcast
    across partitions already; likewise the channel-attn contraction uses a
    replicated lhsT so chan arrives pre-broadcast as [128, 512] chunks.
  - Cross-partition reductions (LN stats, channel-softmax Z) via exact-f32
    gpsimd.partition_all_reduce.
  - relu+mask-weight+reduce fused into one DVE scalar_tensor_tensor per tile.
"""

import sys

sys.path.insert(0, "/opt/trn_rl_repo")

import numpy as np

import concourse.bass as bass  # noqa: F401  (bass must import before bacc)
import concourse.tile as tile
from concourse import bacc, bass_isa, bass_utils, mybir

# Problem constants (hardcoded per contract).
B, C, H, W = 16, 1024, 64, 64
HW = H * W               # 4096
CH = C // 2              # 512
N_CORES = 8
S = B // N_CORES         # 2 samples per core
P = 128                  # SBUF partitions
KT = C // P              # 8 k-tiles over input channels
MT = CH // P             # 4 m-tiles over output channels
NW = 512                 # n-chunk width (one PSUM bank of f32)
NCH = HW // NW           # 8 n-chunks
LN_EPS = 1e-5

F32 = mybir.dt.float32
F32R = mybir.dt.float32r
F8 = mybir.dt.float8e4
Alu = mybir.AluOpType
Act = mybir.ActivationFunctionType
AxX = mybir.AxisListType.X

_cache = {}


def _r(ap):
    """fp32r view of an f32 AP for full-rate PE matmuls."""
    return ap.bitcast(F32R)


def _build():
    nc = bacc.Bacc(
        "TRN2",
        target_bir_lowering=False,
        debug=False,
        num_devices=N_CORES,
        dynamic_dma_scratch_size=512,
    )

    # x: [S, chunk, P, KT, NW] so one chunk is a single DMA with 16KB
    # contiguous per partition; weights likewise partition-major.
    x_d = nc.dram_tensor("x", [S, NCH, P, KT, NW], F32, kind="ExternalInput")
    wvr_d = nc.dram_tensor("wvr", [P, KT, CH], F32, kind="ExternalInput")
    wql_d = nc.dram_tensor("wql", [P, KT, CH], F8, kind="ExternalInput")
    xq_d = nc.dram_tensor("xq", [S, NCH, P, KT, NW], F8, kind="ExternalInput")
    wvl_d = nc.dram_tensor("wvl", [P, KT, CH], F8, kind="ExternalInput")
    wqr_d = nc.dram_tensor("wqr", [P, KT, P], F32, kind="ExternalInput")
    out_d = nc.dram_tensor("out", [S, KT, P, HW], F32, kind="ExternalOutput")

    with tile.TileContext(nc) as tc:
        with (
            tc.tile_pool(name="xp", bufs=NCH) as xp,
            tc.tile_pool(name="wp", bufs=1) as wp,
            tc.tile_pool(name="actp", bufs=2) as actp,
            tc.tile_pool(name="deadp", bufs=1) as deadp,
            tc.tile_pool(name="thp", bufs=4) as thp,
            tc.tile_pool(name="smp", bufs=2) as smp,
            tc.tile_pool(name="erp", bufs=2 * MT) as erp,
            tc.tile_pool(name="x8p", bufs=5) as x8p,
            tc.tile_pool(name="psA", bufs=2, space="PSUM") as psA,
            tc.tile_pool(name="psB", bufs=5, space="PSUM") as psB,
            tc.tile_pool(name="psD", bufs=1, space="PSUM") as psD,
        ):
            # ---- constants ----
            epst = wp.tile([P, 1], F32, name="epst", tag="epst")
            nc.vector.memset(epst[:], LN_EPS)

            # ---- weight tiles: one [P, KT, cols] tile + one DMA per tensor;
            # DMAs are emitted in priority order below so the first qr/vr
            # matmuls are not stuck behind 6MB of wql/wvl ----
            wqr_sb = wp.tile([P, KT, P], F32R, name="wqrsb", tag="wqrsb")
            wvr_sb = wp.tile([P, KT, CH], F32R, name="wvrsb", tag="wvrsb")
            wql_sb = wp.tile([P, KT, CH], F8, name="wqlsb", tag="wqlsb")
            wvl_sb = wp.tile([P, KT, CH], F8, name="wvlsb", tag="wvlsb")
            wdma = {"wqr": wqr_d, "wvr": wvr_d, "wql": wql_d, "wvl": wvl_d}

            def load_w(t, nm):
                s_ap = wdma[nm].ap()[:]
                if t.dtype == F32R:
                    s_ap = s_ap.bitcast(F32R)
                nc.sync.dma_start(t[:], s_ap)

            for s in range(S):
                # ---- load x: chunk-major so chunk 0 is ready first; on the
                # first sample, interleave weight loads by first-use priority
                xc = []
                for i in range(NCH):
                    t = xp.tile([P, KT, NW], F32R, name=f"x{s}_{i}", tag="x")
                    nc.sync.dma_start(t[:], x_d.ap()[s, i].bitcast(F32R))
                    xc.append(t)
                    if s == 0 and i == 0:
                        load_w(wqr_sb, "wqr")
                        load_w(wvr_sb, "wvr")
                    elif s == 0 and i == 2:
                        load_w(wql_sb, "wql")
                    elif s == 0 and i == 3:
                        load_w(wvl_sb, "wvl")
                xt = [[xc[i][:, k, :] for i in range(NCH)] for k in range(KT)]

                # per-sample accumulators
                zpart = smp.tile([P, NCH], F32, name=f"zpart{s}", tag="zpart")
                ctxp = [
                    smp.tile([P, NCH], F32, name=f"ctxp{s}_{m}", tag=f"ctxp{m}")
                    for m in range(MT)
                ]
                gp = [
                    smp.tile([P, NCH], F32, name=f"gp{s}_{m}", tag=f"gp{m}")
                    for m in range(MT)
                ]

                # ---- phase A: qr conv (mask logits) + vr conv (context) ----
                for i in range(NCH):
                    psq = psA.tile([P, NW], F32, name=f"psq{s}_{i}", tag="psA")
                    for k in range(KT):
                        nc.tensor.matmul(
                            psq[:], wqr_sb[:, k, :], xt[k][i],
                            start=(k == 0), stop=(k == KT - 1),
                        )
                    # exp(relu(z)) == max(exp(z), 1): ACT exp, then DVE
                    # in-place max with Z partials via accum
                    et = actp.tile([P, NW], F32, name=f"et{s}_{i}", tag="et")
                    nc.scalar.activation(et[:], psq[:], Act.Exp)
                    nc.vector.tensor_scalar(
                        et[:], et[:], 1.0, 0.0, Alu.max, Alu.add,
                        accum_out=zpart[:, i : i + 1],
                    )
                    for m in range(MT):
                        psv = psB.tile([P, NW], F32, name=f"psv{s}a{i}_{m}", tag="psB")
                        for k in range(KT):
                            nc.tensor.matmul(
                                psv[:],
                                wvr_sb[:, k, m * P : (m + 1) * P],
                                xt[k][i],
                                start=(k == 0), stop=(k == KT - 1),
                            )
                        # ctx partial: sum_n relu(vr) * exp(relu(qr))
                        scr = deadp.tile([P, NW], F32, name=f"sttscr{s}", tag="sttscr")
                        nc.vector.scalar_tensor_tensor(
                            scr[:], psv[:], 0.0, et[:], Alu.max, Alu.mult,
                            accum_out=ctxp[m][:, i : i + 1],
                        )

                # ---- finalize mask Z and context; layernorm + sigmoid -> s ----
                Zt = smp.tile([P, 1], F32, name=f"Z{s}", tag="Z")
                nc.vector.tensor_reduce(Zt[:], zpart[:], AxX, Alu.add)
                rZ = smp.tile([P, 1], F32, name=f"rZ{s}", tag="rZ")
                nc.vector.reciprocal(rZ[:], Zt[:])
                ctx44 = smp.tile([P, MT], F32, name=f"ctx44{s}", tag="ctx44")
                for m in range(MT):
                    cred = smp.tile([P, 1], F32, name=f"cred{s}_{m}", tag="cred")
                    nc.vector.tensor_reduce(cred[:], ctxp[m][:], AxX, Alu.add)
                    nc.vector.tensor_scalar(
                        ctx44[:, m : m + 1], cred[:], rZ[:], None, Alu.mult
                    )
                lnsum = smp.tile([P, MT], F32, name=f"lnsum{s}", tag="lnsum")
                nc.gpsimd.partition_all_reduce(
                    lnsum[:], ctx44[:], P, bass_isa.ReduceOp.add
                )
                tot = smp.tile([P, 1], F32, name=f"tot{s}", tag="tot")
                nc.vector.tensor_reduce(tot[:], lnsum[:], AxX, Alu.add)
                mu = smp.tile([P, 1], F32, name=f"mu{s}", tag="mu")
                nc.vector.tensor_scalar(mu[:], tot[:], 1.0 / CH, None, Alu.mult)
                d44 = smp.tile([P, MT], F32, name=f"d44{s}", tag="d44")
                nc.vector.tensor_scalar(d44[:], ctx44[:], mu[:], None, Alu.subtract)
                d2 = smp.tile([P, MT], F32, name=f"d2{s}", tag="d2")
                nc.vector.tensor_tensor(d2[:], d44[:], d44[:], Alu.mult)
                vsum = smp.tile([P, MT], F32, name=f"vsum{s}", tag="vsum")
                nc.gpsimd.partition_all_reduce(
                    vsum[:], d2[:], P, bass_isa.ReduceOp.add
                )
                vtot = smp.tile([P, 1], F32, name=f"vtot{s}", tag="vtot")
                nc.vector.tensor_reduce(vtot[:], vsum[:], AxX, Alu.add)
                var = smp.tile([P, 1], F32, name=f"var{s}", tag="var")
                nc.vector.tensor_scalar(var[:], vtot[:], 1.0 / CH, None, Alu.mult)
                # ---- phase B: ql conv in fp8 DoubleRow (2 k-tiles/pass) ----
                # x and w_ql are pre-quantized to e4m3 on the host (w scaled
                # by 64; folded back out in the e44 exp scale). Quantization
                # noise washes out through mean(4096) + near-uniform softmax.
                xq_t = {}
                for i in range(NCH - 1, -1, -1):
                    # reverse order: chunks 0..4 stay resident in the 5-slot
                    # pool for phase C to reuse without re-DMA
                    xq = x8p.tile([P, KT, NW], F8, name=f"xq{s}_{i}", tag="xq8")
                    nc.sync.dma_start(xq[:], xq_d.ap()[s, i])
                    xq_t[i] = xq
                    for m in range(MT):
                        psv = psB.tile([P, NW], F32, name=f"psv{s}b{i}_{m}", tag="psB")
                        for a in range(KT // 2):
                            nc.tensor.matmul(
                                psv[:],
                                wql_sb[:, 2 * a : 2 * a + 2, m * P : (m + 1) * P],
                                xq[:, 2 * a : 2 * a + 2, :],
                                start=(a == 0), stop=(a == KT // 2 - 1),
                                perf_mode=mybir.MatmulPerfMode.DoubleRow,
                            )
                        # relu + accumulate mean partials; alternate engines
                        if m % 2 == 0:
                            scr = psD.tile([P, NW], F32, name=f"qlscr{s}", tag="psD")
                            nc.scalar.activation(
                                scr[:], psv[:], Act.Relu, accum_out=gp[m][:, i : i + 1]
                            )
                        else:
                            scr2 = deadp.tile([P, NW], F32, name=f"sttscr{s}b", tag="sttscr")
                            nc.vector.tensor_scalar(
                                scr2[:], psv[:], 0.0, 0.0, Alu.max, Alu.add,
                                accum_out=gp[m][:, i : i + 1],
                            )
                g44 = smp.tile([P, MT], F32, name=f"g44{s}", tag="g44")
                for m in range(MT):
                    nc.vector.tensor_reduce(g44[:, m : m + 1], gp[m][:], AxX, Alu.add)
                e44 = smp.tile([P, MT], F32, name=f"e44{s}", tag="e44")
                nc.scalar.activation(e44[:], g44[:], Act.Exp, scale=1.0 / (HW * 64.0))
                std = smp.tile([P, 1], F32, name=f"std{s}", tag="std")
                nc.scalar.activation(std[:], var[:], Act.Sqrt, bias=epst[:])
                rstd = smp.tile([P, 1], F32, name=f"rstd{s}", tag="rstd")
                nc.vector.reciprocal(rstd[:], std[:])
                spre = smp.tile([P, MT], F32, name=f"spre{s}", tag="spre")
                nc.vector.tensor_scalar(
                    spre[:], ctx44[:], mu[:], rstd[:], Alu.subtract, Alu.mult
                )
                s44 = smp.tile([P, MT], F32, name=f"s44{s}", tag="s44")
                nc.scalar.activation(s44[:], spre[:], Act.Sigmoid)
                sp44 = smp.tile([P, MT], F32, name=f"sp44{s}", tag="sp44")
                nc.vector.tensor_scalar(sp44[:], s44[:], 1.0, None, Alu.add)

                ze = smp.tile([P, MT], F32, name=f"ze{s}", tag="ze")
                nc.gpsimd.partition_all_reduce(ze[:], e44[:], P, bass_isa.ReduceOp.add)
                zet = smp.tile([P, 1], F32, name=f"zet{s}", tag="zet")
                nc.vector.tensor_reduce(zet[:], ze[:], AxX, Alu.add)
                rZc = smp.tile([P, 1], F32, name=f"rZc{s}", tag="rZc")
                nc.vector.reciprocal(rZc[:], zet[:])
                erep = []
                for m in range(MT):
                    er = erp.tile([P, P], F32R, name=f"erep{s}_{m}", tag="erep")
                    # 1/64 compensates the x64 fp8 scaling of wvl
                    nc.vector.tensor_scalar(
                        er[:], e44[:, m : m + 1].broadcast_to([P, P]),
                        1.0 / 64.0, None, Alu.mult,
                    )
                    erep.append(er)

                # ---- phase C: vl conv -> chan attn -> finale + store ----
                for i in range(NCH):
                    if i + 5 < NCH:
                        # prefetch fp8 chunks evicted by the reverse phase-B
                        xq = x8p.tile([P, KT, NW], F8, name=f"xqc{s}_{i + 5}", tag="xq8")
                        nc.sync.dma_start(xq[:], xq_d.ap()[s, i + 5])
                        xq_t[i + 5] = xq
                    pschan = psA.tile([P, NW], F32, name=f"psc{s}_{i}", tag="psA")
                    thl = []
                    for m in range(MT):
                        psv = psB.tile([P, NW], F32, name=f"psv{s}c{i}_{m}", tag="psB")
                        for a in range(KT // 2):
                            nc.tensor.matmul(
                                psv[:],
                                wvl_sb[:, 2 * a : 2 * a + 2, m * P : (m + 1) * P],
                                xq_t[i][:, 2 * a : 2 * a + 2, :],
                                start=(a == 0), stop=(a == KT // 2 - 1),
                                perf_mode=mybir.MatmulPerfMode.DoubleRow,
                            )
                        th = thp.tile([P, NW], F32R, name=f"th{s}_{i}_{m}", tag="th")
                        nc.scalar.activation(th[:], psv[:], Act.Relu)
                        thl.append(th)
                    # chan partials after all relus so the PE stalls at most on
                    # the last one: rows of pschan are broadcast copies of
                    # sum_c e_g[c] * theta[c, n]
                    for m in range(MT):
                        nc.tensor.matmul(
                            pschan[:], erep[m][:], thl[m][:],
                            start=(m == 0), stop=(m == MT - 1),
                            skip_group_check=True,
                        )
                    chant = actp.tile([P, NW], F32, name=f"ch{s}_{i}", tag="chant")
                    nc.scalar.activation(chant[:], pschan[:], Act.Sigmoid, scale=rZc[:])
                    # finale: seq rows k<4: x*(1 + s*chan); par rows: x*(chan+1+s).
                    # Work is spread across ACT/DVE/GpSimd (~5us per chunk each).
                    for k in range(KT):
                        xf = xt[k][i].bitcast(F32)
                        ot = actp.tile([P, NW], F32, name=f"ot{s}_{i}_{k}", tag="a1", bufs=6)
                        if k < 2:
                            # attn on ACT, mult on DVE
                            nc.scalar.activation(
                                ot[:], chant[:], Act.Copy,
                                scale=s44[:, k : k + 1], bias=1.0,
                            )
                            nc.vector.tensor_tensor(ot[:], ot[:], xf, Alu.mult)
                        elif k < MT:
                            # attn on GpSimd, mult on DVE
                            nc.gpsimd.tensor_scalar(
                                ot[:], chant[:], s44[:, k : k + 1], 1.0,
                                Alu.mult, Alu.add,
                            )
                            nc.vector.tensor_tensor(ot[:], ot[:], xf, Alu.mult)
                        elif k < 6:
                            # fused attn+mult on DVE
                            nc.vector.scalar_tensor_tensor(
                                ot[:], chant[:], sp44[:, k - MT : k - MT + 1],
                                xf, Alu.add, Alu.mult,
                            )
                        else:
                            # attn + mult both on GpSimd
                            nc.gpsimd.tensor_scalar(
                                ot[:], chant[:], sp44[:, k - MT : k - MT + 1],
                                None, Alu.add,
                            )
                            nc.gpsimd.tensor_tensor(ot[:], ot[:], xf, Alu.mult)
                        nc.sync.dma_start(
                            out_d.ap()[s, k, :, i * NW : (i + 1) * NW], ot[:]
                        )

    nc.compile()
    return nc


def _prep_inputs(x, w_qr, w_vr, w_ql, w_vl):
    x = np.asarray(x, dtype=np.float32).reshape(B, C, HW)
    wts = {}
    for nm, w in (("wvr", w_vr), ("wql", w_ql), ("wvl", w_vl)):
        w = np.asarray(w, dtype=np.float32)
        # (out, in) -> [P, KT, out]: wts[nm][p, k, o] = w[o, 128k + p]
        wts[nm] = np.ascontiguousarray(w.T.reshape(KT, P, CH).transpose(1, 0, 2))
    q = np.asarray(w_qr, dtype=np.float32).reshape(KT, P).T  # [P, KT]
    wts["wqr"] = np.ascontiguousarray(np.broadcast_to(q[:, :, None], (P, KT, P)))
    # ql runs in fp8e4m3 DoubleRow; scale weights x64 into fp8 range (the
    # 1/64 is folded into the e44 exp scale)
    import ml_dtypes

    f8 = np.dtype(ml_dtypes.float8_e4m3)
    wts["wql"] = (wts["wql"] * 64.0).astype(f8)
    wts["wvl"] = (wts["wvl"] * 64.0).astype(f8)
    in_maps = []
    for c in range(N_CORES):
        m = dict(wts)
        # [S, chunk, P, KT, NW]: m["x"][s, i, p, k, n] = x[s, 128k+p, 512i+n]
        m["x"] = np.ascontiguousarray(
            x[S * c : S * (c + 1)]
            .reshape(S, KT, P, NCH, NW)
            .transpose(0, 3, 2, 1, 4)
        )
        m["xq"] = m["x"].astype(f8)
        in_maps.append(m)
    return in_maps


def _run(x, w_qr, w_vr, w_ql, w_vl, trace=False):
    if "nc" not in _cache:
        _cache["nc"] = _build()
    nc = _cache["nc"]
    in_maps = _prep_inputs(x, w_qr, w_vr, w_ql, w_vl)
    res = bass_utils.run_bass_kernel_spmd(
        nc, in_maps, core_ids=list(range(N_CORES)), trace=trace
    )
    out = np.empty((B, C, HW), np.float32)
    for c in range(N_CORES):
        out[S * c : S * (c + 1)] = res.results[c]["out"].reshape(S, C, HW)
    return out.reshape(B, C, H, W), res


def kernel(x, w_qr, w_vr, w_ql, w_vl):
    out, _ = _run(x, w_qr, w_vr, w_ql, w_vl, trace=False)
    return out


# revision 22
# speedup vs baseline: 1.6484x; 1.6484x over previous
"""Trainium2 Bass kernel for the dual-attention module (spatial + channel attention).

Contract: kernel(**inputs) takes the FULL inputs (x: (16,1024,64,64) f32 plus four
1x1-conv weight matrices) and returns the FULL output (16,1024,64,64) f32.
Internally shards data-parallel over batch across 8 NeuronCores (2 samples/core),
weights replicated.

Per-sample math (b, c=1024, ch=512, hw=4096):
  conv(w) = relu(w @ X)               X = x[b] as (1024, 4096)
  mask    = softmax(conv(w_qr))       over hw          (spatial attn branch)
  ctx     = conv(w_vr) @ mask         (ch,)
  s       = sigmoid(layernorm(ctx))   (ch,)
  avg     = softmax(mean_hw(conv(w_ql)))               (channel attn branch)
  chan    = sigmoid(avg @ conv(w_vl)) (hw,)
  out[0:512]    = x * (1 + s*chan)                     ("sequence")
  out[512:1024] = x * (1 + s + chan)                   ("parallel")

Kernel strategy per core:
  - x sample resident in SBUF as 64 [128,512] f32 tiles (8 k-tiles x 8 n-chunks).
  - All matmuls in fp32r (full PE rate at N=512).
  - Softmaxes are computed unnormalized (exp only); the 1/Z scales are folded
    into later scalar multiplies, so no [1,N] single-partition ops anywhere.
  - qr conv uses a column-replicated weight so its psum rows are broadcast
    across partitions already; likewise the channel-attn contraction uses a
    replicated lhsT so chan arrives pre-broadcast as [128, 512] chunks.
  - Cross-partition reductions (LN stats, channel-softmax Z) via exact-f32
    gpsimd.partition_all_reduce.
  - relu+mask-weight+reduce fused into one DVE scalar_tensor_tensor per tile.
"""

import sys

sys.path.insert(0, "/opt/trn_rl_repo")

import numpy as np

import concourse.bass as bass  # noqa: F401  (bass must import before bacc)
import concourse.tile as tile
from concourse import bacc, bass_isa, bass_utils, mybir

# Problem constants (hardcoded per contract).
B, C, H, W = 16, 1024, 64, 64
HW = H * W               # 4096
CH = C // 2              # 512
N_CORES = 8
S = B // N_CORES         # 2 samples per core
P = 128                  # SBUF partitions
KT = C // P              # 8 k-tiles over input channels
MT = CH // P             # 4 m-tiles over output channels
NW = 512                 # n-chunk width (one PSUM bank of f32)
NCH = HW // NW           # 8 n-chunks
LN_EPS = 1e-5

F32 = mybir.dt.float32
F32R = mybir.dt.float32r
F8 = mybir.dt.float8e4
Alu = mybir.AluOpType
Act = mybir.ActivationFunctionType
AxX = mybir.AxisListType.X

_cache = {}


def _r(ap):
    """fp32r view of an f32 AP for full-rate PE matmuls."""
    return ap.bitcast(F32R)


def _build():
    nc = bacc.Bacc(
        "TRN2",
        target_bir_lowering=False,
        debug=False,
        num_devices=N_CORES,
        dynamic_dma_scratch_size=512,
    )

    # x: [S, chunk, P, KT, NW] so one chunk is a single DMA with 16KB
    # contiguous per partition; weights likewise partition-major.
    x_d = nc.dram_tensor("x", [S, NCH, P, KT, NW], F32, kind="ExternalInput")
    wvr_d = nc.dram_tensor("wvr", [P, KT, CH], F32, kind="ExternalInput")
    wql_d = nc.dram_tensor("wql", [P, KT, CH], F8, kind="ExternalInput")
    xq_d = nc.dram_tensor("xq", [S, NCH, P, KT, NW], F8, kind="ExternalInput")
    wvl_d = nc.dram_tensor("wvl", [P, KT, CH], F8, kind="ExternalInput")
    wqr_d = nc.dram_tensor("wqr", [P, KT, P], F32, kind="ExternalInput")
    out_d = nc.dram_tensor("out", [S, KT, P, HW], F32, kind="ExternalOutput")

    with tile.TileContext(nc) as tc:
        with (
            tc.tile_pool(name="xp", bufs=NCH) as xp,
            tc.tile_pool(name="wp", bufs=1) as wp,
            tc.tile_pool(name="actp", bufs=2) as actp,
            tc.tile_pool(name="deadp", bufs=1) as deadp,
            tc.tile_pool(name="thp", bufs=4) as thp,
            tc.tile_pool(name="smp", bufs=2) as smp,
            tc.tile_pool(name="erp", bufs=2 * MT) as erp,
            tc.tile_pool(name="x8p", bufs=5) as x8p,
            tc.tile_pool(name="psA", bufs=3, space="PSUM") as psA,
            tc.tile_pool(name="psB", bufs=4, space="PSUM") as psB,
            tc.tile_pool(name="psD", bufs=1, space="PSUM") as psD,
        ):
            # ---- constants ----
            epst = wp.tile([P, 1], F32, name="epst", tag="epst")
            nc.vector.memset(epst[:], LN_EPS)

            # ---- weight tiles: one [P, KT, cols] tile + one DMA per tensor;
            # DMAs are emitted in priority order below so the first qr/vr
            # matmuls are not stuck behind 6MB of wql/wvl ----
            wqr_sb = wp.tile([P, KT, P], F32R, name="wqrsb", tag="wqrsb")
            wvr_sb = wp.tile([P, KT, CH], F32R, name="wvrsb", tag="wvrsb")
            wql_sb = wp.tile([P, KT, CH], F8, name="wqlsb", tag="wqlsb")
            wvl_sb = wp.tile([P, KT, CH], F8, name="wvlsb", tag="wvlsb")
            wdma = {"wqr": wqr_d, "wvr": wvr_d, "wql": wql_d, "wvl": wvl_d}

            def load_w(t, nm):
                s_ap = wdma[nm].ap()[:]
                if t.dtype == F32R:
                    s_ap = s_ap.bitcast(F32R)
                nc.sync.dma_start(t[:], s_ap)

            for s in range(S):
                # ---- load x: chunk-major so chunk 0 is ready first; on the
                # first sample, interleave weight loads by first-use priority
                xc = []
                for i in range(NCH):
                    t = xp.tile([P, KT, NW], F32R, name=f"x{s}_{i}", tag="x")
                    nc.sync.dma_start(t[:], x_d.ap()[s, i].bitcast(F32R))
                    xc.append(t)
                    if s == 0 and i == 0:
                        load_w(wqr_sb, "wqr")
                        load_w(wvr_sb, "wvr")
                    elif s == 0 and i == 2:
                        load_w(wql_sb, "wql")
                    elif s == 0 and i == 3:
                        load_w(wvl_sb, "wvl")
                xt = [[xc[i][:, k, :] for i in range(NCH)] for k in range(KT)]

                # per-sample accumulators
                zpart = smp.tile([P, NCH], F32, name=f"zpart{s}", tag="zpart")
                ctxp = [
                    smp.tile([P, NCH], F32, name=f"ctxp{s}_{m}", tag=f"ctxp{m}")
                    for m in range(MT)
                ]
                gp = [
                    smp.tile([P, NCH], F32, name=f"gp{s}_{m}", tag=f"gp{m}")
                    for m in range(MT)
                ]

                # ---- phase A: qr conv (mask logits) + vr conv (context) ----
                for i in range(NCH):
                    psq = psA.tile([P, NW], F32, name=f"psq{s}_{i}", tag="psA")
                    for k in range(KT):
                        nc.tensor.matmul(
                            psq[:], wqr_sb[:, k, :], xt[k][i],
                            start=(k == 0), stop=(k == KT - 1),
                        )
                    # exp(relu(z)) == max(exp(z), 1): ACT exp, then DVE
                    # in-place max with Z partials via accum
                    et = actp.tile([P, NW], F32, name=f"et{s}_{i}", tag="et")
                    nc.scalar.activation(et[:], psq[:], Act.Exp)
                    nc.vector.tensor_scalar(
                        et[:], et[:], 1.0, 0.0, Alu.max, Alu.add,
                        accum_out=zpart[:, i : i + 1],
                    )
                    for m in range(MT):
                        psv = psB.tile([P, NW], F32, name=f"psv{s}a{i}_{m}", tag="psB")
                        for k in range(KT):
                            nc.tensor.matmul(
                                psv[:],
                                wvr_sb[:, k, m * P : (m + 1) * P],
                                xt[k][i],
                                start=(k == 0), stop=(k == KT - 1),
                            )
                        # ctx partial: sum_n relu(vr) * exp(relu(qr))
                        scr = deadp.tile([P, NW], F32, name=f"sttscr{s}", tag="sttscr")
                        nc.vector.scalar_tensor_tensor(
                            scr[:], psv[:], 0.0, et[:], Alu.max, Alu.mult,
                            accum_out=ctxp[m][:, i : i + 1],
                        )

                # ---- finalize mask Z and context; layernorm + sigmoid -> s ----
                Zt = smp.tile([P, 1], F32, name=f"Z{s}", tag="Z")
                nc.vector.tensor_reduce(Zt[:], zpart[:], AxX, Alu.add)
                rZ = smp.tile([P, 1], F32, name=f"rZ{s}", tag="rZ")
                nc.vector.reciprocal(rZ[:], Zt[:])
                ctx44 = smp.tile([P, MT], F32, name=f"ctx44{s}", tag="ctx44")
                for m in range(MT):
                    cred = smp.tile([P, 1], F32, name=f"cred{s}_{m}", tag="cred")
                    nc.vector.tensor_reduce(cred[:], ctxp[m][:], AxX, Alu.add)
                    nc.vector.tensor_scalar(
                        ctx44[:, m : m + 1], cred[:], rZ[:], None, Alu.mult
                    )
                lnsum = smp.tile([P, MT], F32, name=f"lnsum{s}", tag="lnsum")
                nc.gpsimd.partition_all_reduce(
                    lnsum[:], ctx44[:], P, bass_isa.ReduceOp.add
                )
                tot = smp.tile([P, 1], F32, name=f"tot{s}", tag="tot")
                nc.vector.tensor_reduce(tot[:], lnsum[:], AxX, Alu.add)
                mu = smp.tile([P, 1], F32, name=f"mu{s}", tag="mu")
                nc.vector.tensor_scalar(mu[:], tot[:], 1.0 / CH, None, Alu.mult)
                d44 = smp.tile([P, MT], F32, name=f"d44{s}", tag="d44")
                nc.vector.tensor_scalar(d44[:], ctx44[:], mu[:], None, Alu.subtract)
                d2 = smp.tile([P, MT], F32, name=f"d2{s}", tag="d2")
                nc.vector.tensor_tensor(d2[:], d44[:], d44[:], Alu.mult)
                vsum = smp.tile([P, MT], F32, name=f"vsum{s}", tag="vsum")
                nc.gpsimd.partition_all_reduce(
                    vsum[:], d2[:], P, bass_isa.ReduceOp.add
                )
                vtot = smp.tile([P, 1], F32, name=f"vtot{s}", tag="vtot")
                nc.vector.tensor_reduce(vtot[:], vsum[:], AxX, Alu.add)
                var = smp.tile([P, 1], F32, name=f"var{s}", tag="var")
                nc.vector.tensor_scalar(var[:], vtot[:], 1.0 / CH, None, Alu.mult)
                # ---- phase B: ql conv in fp8 DoubleRow (2 k-tiles/pass) ----
                # x and w_ql are pre-quantized to e4m3 on the host (w scaled
                # by 64; folded back out in the e44 exp scale). Quantization
                # noise washes out through mean(4096) + near-uniform softmax.
                xq_t = {}
                for i in range(NCH - 1, -1, -1):
                    # reverse order: chunks 0..4 stay resident in the 5-slot
                    # pool for phase C to reuse without re-DMA
                    xq = x8p.tile([P, KT, NW], F8, name=f"xq{s}_{i}", tag="xq8")
                    nc.sync.dma_start(xq[:], xq_d.ap()[s, i])
                    xq_t[i] = xq
                    for m in range(MT):
                        psv = psB.tile([P, NW], F32, name=f"psv{s}b{i}_{m}", tag="psB")
                        for a in range(KT // 2):
                            nc.tensor.matmul(
                                psv[:],
                                wql_sb[:, 2 * a : 2 * a + 2, m * P : (m + 1) * P],
                                xq[:, 2 * a : 2 * a + 2, :],
                                start=(a == 0), stop=(a == KT // 2 - 1),
                                perf_mode=mybir.MatmulPerfMode.DoubleRow,
                            )
                        # relu + accumulate mean partials; alternate engines
                        if m % 2 == 0:
                            scr = psD.tile([P, NW], F32, name=f"qlscr{s}", tag="psD")
                            nc.scalar.activation(
                                scr[:], psv[:], Act.Relu, accum_out=gp[m][:, i : i + 1]
                            )
                        else:
                            scr2 = deadp.tile([P, NW], F32, name=f"sttscr{s}b", tag="sttscr")
                            nc.vector.tensor_scalar(
                                scr2[:], psv[:], 0.0, 0.0, Alu.max, Alu.add,
                                accum_out=gp[m][:, i : i + 1],
                            )
                g44 = smp.tile([P, MT], F32, name=f"g44{s}", tag="g44")
                for m in range(MT):
                    nc.vector.tensor_reduce(g44[:, m : m + 1], gp[m][:], AxX, Alu.add)
                e44 = smp.tile([P, MT], F32, name=f"e44{s}", tag="e44")
                nc.scalar.activation(e44[:], g44[:], Act.Exp, scale=1.0 / (HW * 64.0))
                std = smp.tile([P, 1], F32, name=f"std{s}", tag="std")
                nc.scalar.activation(std[:], var[:], Act.Sqrt, bias=epst[:])
                rstd = smp.tile([P, 1], F32, name=f"rstd{s}", tag="rstd")
                nc.vector.reciprocal(rstd[:], std[:])
                spre = smp.tile([P, MT], F32, name=f"spre{s}", tag="spre")
                nc.vector.tensor_scalar(
                    spre[:], ctx44[:], mu[:], rstd[:], Alu.subtract, Alu.mult
                )
                s44 = smp.tile([P, MT], F32, name=f"s44{s}", tag="s44")
                nc.scalar.activation(s44[:], spre[:], Act.Sigmoid)
                sp44 = smp.tile([P, MT], F32, name=f"sp44{s}", tag="sp44")
                nc.vector.tensor_scalar(sp44[:], s44[:], 1.0, None, Alu.add)

                ze = smp.tile([P, MT], F32, name=f"ze{s}", tag="ze")
                nc.gpsimd.partition_all_reduce(ze[:], e44[:], P, bass_isa.ReduceOp.add)
                zet = smp.tile([P, 1], F32, name=f"zet{s}", tag="zet")
                nc.vector.tensor_reduce(zet[:], ze[:], AxX, Alu.add)
                rZc = smp.tile([P, 1], F32, name=f"rZc{s}", tag="rZc")
                nc.vector.reciprocal(rZc[:], zet[:])
                erep = []
                for m in range(MT):
                    er = erp.tile([P, P], F32R, name=f"erep{s}_{m}", tag="erep")
                    # 1/64 compensates the x64 fp8 scaling of wvl
                    nc.vector.tensor_scalar(
                        er[:], e44[:, m : m + 1].broadcast_to([P, P]),
                        1.0 / 64.0, None, Alu.mult,
                    )
                    erep.append(er)

                # ---- phase C: vl conv -> chan attn -> finale + store ----
                for i in range(NCH):
                    if i + 5 < NCH:
                        # prefetch fp8 chunks evicted by the reverse phase-B
                        xq = x8p.tile([P, KT, NW], F8, name=f"xqc{s}_{i + 5}", tag="xq8")
                        nc.sync.dma_start(xq[:], xq_d.ap()[s, i + 5])
                        xq_t[i + 5] = xq
                    pschan = psA.tile([P, NW], F32, name=f"psc{s}_{i}", tag="psA")
                    thl = []
                    for m in range(MT):
                        psv = psB.tile([P, NW], F32, name=f"psv{s}c{i}_{m}", tag="psB")
                        for a in range(KT // 2):
                            nc.tensor.matmul(
                                psv[:],
                                wvl_sb[:, 2 * a : 2 * a + 2, m * P : (m + 1) * P],
                                xq_t[i][:, 2 * a : 2 * a + 2, :],
                                start=(a == 0), stop=(a == KT // 2 - 1),
                                perf_mode=mybir.MatmulPerfMode.DoubleRow,
                            )
                        th = thp.tile([P, NW], F32R, name=f"th{s}_{i}_{m}", tag="th")
                        nc.scalar.activation(th[:], psv[:], Act.Relu)
                        thl.append(th)
                    # chan partials after all relus so the PE stalls at most on
                    # the last one: rows of pschan are broadcast copies of
                    # sum_c e_g[c] * theta[c, n]
                    for m in range(MT):
                        nc.tensor.matmul(
                            pschan[:], erep[m][:], thl[m][:],
                            start=(m == 0), stop=(m == MT - 1),
                            skip_group_check=True,
                        )
                    chant = actp.tile([P, NW], F32, name=f"ch{s}_{i}", tag="chant", bufs=4)
                    nc.scalar.activation(chant[:], pschan[:], Act.Sigmoid, scale=rZc[:])
                    # finale: seq rows k<4: x*(1 + s*chan); par rows: x*(chan+1+s).
                    # Work is spread across ACT/DVE/GpSimd (~5us per chunk each).
                    for k in range(KT):
                        xf = xt[k][i].bitcast(F32)
                        ot = actp.tile([P, NW], F32, name=f"ot{s}_{i}_{k}", tag="a1", bufs=6)
                        if k < 2:
                            # attn on ACT, mult on DVE
                            nc.scalar.activation(
                                ot[:], chant[:], Act.Copy,
                                scale=s44[:, k : k + 1], bias=1.0,
                            )
                            nc.vector.tensor_tensor(ot[:], ot[:], xf, Alu.mult)
                        elif k < MT:
                            # attn on GpSimd, mult on DVE
                            nc.gpsimd.tensor_scalar(
                                ot[:], chant[:], s44[:, k : k + 1], 1.0,
                                Alu.mult, Alu.add,
                            )
                            nc.vector.tensor_tensor(ot[:], ot[:], xf, Alu.mult)
                        else:
                            # fused attn+mult on DVE
                            nc.vector.scalar_tensor_tensor(
                                ot[:], chant[:], sp44[:, k - MT : k - MT + 1],
                                xf, Alu.add, Alu.mult,
                            )
                        nc.sync.dma_start(
                            out_d.ap()[s, k, :, i * NW : (i + 1) * NW], ot[:]
                        )

    nc.compile()
    return nc


def _prep_inputs(x, w_qr, w_vr, w_ql, w_vl):
    x = np.asarray(x, dtype=np.float32).reshape(B, C, HW)
    wts = {}
    for nm, w in (("wvr", w_vr), ("wql", w_ql), ("wvl", w_vl)):
        w = np.asarray(w, dtype=np.float32)
        # (out, in) -> [P, KT, out]: wts[nm][p, k, o] = w[o, 128k + p]
        wts[nm] = np.ascontiguousarray(w.T.reshape(KT, P, CH).transpose(1, 0, 2))
    q = np.asarray(w_qr, dtype=np.float32).reshape(KT, P).T  # [P, KT]
    wts["wqr"] = np.ascontiguousarray(np.broadcast_to(q[:, :, None], (P, KT, P)))
    # ql runs in fp8e4m3 DoubleRow; scale weights x64 into fp8 range (the
    # 1/64 is folded into the e44 exp scale)
    import ml_dtypes

    f8 = np.dtype(ml_dtypes.float8_e4m3)
    wts["wql"] = (wts["wql"] * 64.0).astype(f8)
    wts["wvl"] = (wts["wvl"] * 64.0).astype(f8)
    in_maps = []
    for c in range(N_CORES):
        m = dict(wts)
        # [S, chunk, P, KT, NW]: m["x"][s, i, p, k, n] = x[s, 128k+p, 512i+n]
        m["x"] = np.ascontiguousarray(
            x[S * c : S * (c + 1)]
            .reshape(S, KT, P, NCH, NW)
            .transpose(0, 3, 2, 1, 4)
        )
        m["xq"] = m["x"].astype(f8)
        in_maps.append(m)
    return in_maps


def _run(x, w_qr, w_vr, w_ql, w_vl, trace=False):
    if "nc" not in _cache:
        _cache["nc"] = _build()
    nc = _cache["nc"]
    in_maps = _prep_inputs(x, w_qr, w_vr, w_ql, w_vl)
    res = bass_utils.run_bass_kernel_spmd(
        nc, in_maps, core_ids=list(range(N_CORES)), trace=trace
    )
    out = np.empty((B, C, HW), np.float32)
    for c in range(N_CORES):
        out[S * c : S * (c + 1)] = res.results[c]["out"].reshape(S, C, HW)
    return out.reshape(B, C, H, W), res


def kernel(x, w_qr, w_vr, w_ql, w_vl):
    out, _ = _run(x, w_qr, w_vr, w_ql, w_vl, trace=False)
    return out


# revision 25
# speedup vs baseline: 1.7769x; 1.0779x over previous
"""Trainium2 Bass kernel for the dual-attention module (spatial + channel attention).

Contract: kernel(**inputs) takes the FULL inputs (x: (16,1024,64,64) f32 plus four
1x1-conv weight matrices) and returns the FULL output (16,1024,64,64) f32.
Internally shards data-parallel over batch across 8 NeuronCores (2 samples/core),
weights replicated.

Per-sample math (b, c=1024, ch=512, hw=4096):
  conv(w) = relu(w @ X)               X = x[b] as (1024, 4096)
  mask    = softmax(conv(w_qr))       over hw          (spatial attn branch)
  ctx     = conv(w_vr) @ mask         (ch,)
  s       = sigmoid(layernorm(ctx))   (ch,)
  avg     = softmax(mean_hw(conv(w_ql)))               (channel attn branch)
  chan    = sigmoid(avg @ conv(w_vl)) (hw,)
  out[0:512]    = x * (1 + s*chan)                     ("sequence")
  out[512:1024] = x * (1 + s + chan)                   ("parallel")

Kernel strategy per core:
  - x sample resident in SBUF as 64 [128,512] f32 tiles (8 k-tiles x 8 n-chunks).
  - All matmuls in fp32r (full PE rate at N=512).
  - Softmaxes are computed unnormalized (exp only); the 1/Z scales are folded
    into later scalar multiplies, so no [1,N] single-partition ops anywhere.
  - qr conv uses a column-replicated weight so its psum rows are broadcast
    across partitions already; likewise the channel-attn contraction uses a
    replicated lhsT so chan arrives pre-broadcast as [128, 512] chunks.
  - Cross-partition reductions (LN stats, channel-softmax Z) via exact-f32
    gpsimd.partition_all_reduce.
  - relu+mask-weight+reduce fused into one DVE scalar_tensor_tensor per tile.
"""

import sys

sys.path.insert(0, "/opt/trn_rl_repo")

import numpy as np

import concourse.bass as bass  # noqa: F401  (bass must import before bacc)
import concourse.tile as tile
from concourse import bacc, bass_isa, bass_utils, mybir

# Problem constants (hardcoded per contract).
B, C, H, W = 16, 1024, 64, 64
HW = H * W               # 4096
CH = C // 2              # 512
N_CORES = 8
S = B // N_CORES         # 2 samples per core
P = 128                  # SBUF partitions
KT = C // P              # 8 k-tiles over input channels
MT = CH // P             # 4 m-tiles over output channels
NW = 512                 # n-chunk width (one PSUM bank of f32)
NCH = HW // NW           # 8 n-chunks
LN_EPS = 1e-5

F32 = mybir.dt.float32
F32R = mybir.dt.float32r
F8 = mybir.dt.float8e4
Alu = mybir.AluOpType
Act = mybir.ActivationFunctionType
AxX = mybir.AxisListType.X

_cache = {}


def _r(ap):
    """fp32r view of an f32 AP for full-rate PE matmuls."""
    return ap.bitcast(F32R)


def _build():
    nc = bacc.Bacc(
        "TRN2",
        target_bir_lowering=False,
        debug=False,
        num_devices=N_CORES,
        dynamic_dma_scratch_size=512,
    )

    # x: [S, chunk, P, KT, NW] so one chunk is a single DMA with 16KB
    # contiguous per partition; weights likewise partition-major.
    x_d = nc.dram_tensor("x", [S, NCH, P, KT, NW], F32, kind="ExternalInput")
    wvr_d = nc.dram_tensor("wvr", [P, KT, CH], F32, kind="ExternalInput")
    wql_d = nc.dram_tensor("wql", [P, KT, CH], F8, kind="ExternalInput")
    xq_d = nc.dram_tensor("xq", [S, NCH, P, KT, NW], F8, kind="ExternalInput")
    wvl_d = nc.dram_tensor("wvl", [P, KT, CH], F8, kind="ExternalInput")
    wqr_d = nc.dram_tensor("wqr", [P, KT, P], F32, kind="ExternalInput")
    out_d = nc.dram_tensor("out", [S, KT, P, HW], F32, kind="ExternalOutput")

    with tile.TileContext(nc) as tc:
        with (
            tc.tile_pool(name="xp", bufs=NCH) as xp,
            tc.tile_pool(name="wp", bufs=1) as wp,
            tc.tile_pool(name="actp", bufs=2) as actp,
            tc.tile_pool(name="deadp", bufs=1) as deadp,
            tc.tile_pool(name="thp", bufs=4) as thp,
            tc.tile_pool(name="smp", bufs=2) as smp,
            tc.tile_pool(name="erp", bufs=2 * MT) as erp,
            tc.tile_pool(name="x8p", bufs=5) as x8p,
            tc.tile_pool(name="psA", bufs=3, space="PSUM") as psA,
            tc.tile_pool(name="psB", bufs=4, space="PSUM") as psB,
            tc.tile_pool(name="psD", bufs=1, space="PSUM") as psD,
        ):
            # ---- constants ----
            epst = wp.tile([P, 1], F32, name="epst", tag="epst")
            nc.vector.memset(epst[:], LN_EPS)

            # ---- weight tiles: one [P, KT, cols] tile + one DMA per tensor;
            # DMAs are emitted in priority order below so the first qr/vr
            # matmuls are not stuck behind 6MB of wql/wvl ----
            wqr_sb = wp.tile([P, KT, P], F32R, name="wqrsb", tag="wqrsb")
            wvr_sb = wp.tile([P, KT, CH], F32R, name="wvrsb", tag="wvrsb")
            wql_sb = wp.tile([P, KT, CH], F8, name="wqlsb", tag="wqlsb")
            wvl_sb = wp.tile([P, KT, CH], F8, name="wvlsb", tag="wvlsb")
            wdma = {"wqr": wqr_d, "wvr": wvr_d, "wql": wql_d, "wvl": wvl_d}

            def load_w(t, nm):
                s_ap = wdma[nm].ap()[:]
                if t.dtype == F32R:
                    s_ap = s_ap.bitcast(F32R)
                nc.sync.dma_start(t[:], s_ap)

            def emit_x_load(s_, i_):
                t = xp.tile([P, KT, NW], F32R, name=f"x{s_}_{i_}", tag="x")
                nc.sync.dma_start(t[:], x_d.ap()[s_, i_].bitcast(F32R))
                return t

            # sample-0 x loads up front, weight loads interleaved by first use.
            # Later samples' loads are emitted inside the previous sample's
            # phase C so their Sync-queue dispatch isn't head-of-line blocked
            # behind that sample's out-stores.
            xc_all = {0: []}
            for i in range(NCH):
                xc_all[0].append(emit_x_load(0, i))
                if i == 0:
                    load_w(wqr_sb, "wqr")
                    load_w(wvr_sb, "wvr")
                elif i == 2:
                    load_w(wql_sb, "wql")
                elif i == 3:
                    load_w(wvl_sb, "wvl")

            for s in range(S):
                xc = xc_all[s]
                xt = [[xc[i][:, k, :] for i in range(NCH)] for k in range(KT)]

                # per-sample accumulators
                zpart = smp.tile([P, NCH], F32, name=f"zpart{s}", tag="zpart")
                ctxp = [
                    smp.tile([P, NCH], F32, name=f"ctxp{s}_{m}", tag=f"ctxp{m}")
                    for m in range(MT)
                ]
                gp = [
                    smp.tile([P, NCH], F32, name=f"gp{s}_{m}", tag=f"gp{m}")
                    for m in range(MT)
                ]

                # ---- phase A: qr conv (mask logits) + vr conv (context) ----
                for i in range(NCH):
                    psq = psA.tile([P, NW], F32, name=f"psq{s}_{i}", tag="psA")
                    for k in range(KT):
                        nc.tensor.matmul(
                            psq[:], wqr_sb[:, k, :], xt[k][i],
                            start=(k == 0), stop=(k == KT - 1),
                        )
                    # exp(relu(z)) == max(exp(z), 1): ACT exp, then DVE
                    # in-place max with Z partials via accum
                    et = actp.tile([P, NW], F32, name=f"et{s}_{i}", tag="et")
                    nc.scalar.activation(et[:], psq[:], Act.Exp)
                    nc.vector.tensor_scalar(
                        et[:], et[:], 1.0, 0.0, Alu.max, Alu.add,
                        accum_out=zpart[:, i : i + 1],
                    )
                    for m in range(MT):
                        psv = psB.tile([P, NW], F32, name=f"psv{s}a{i}_{m}", tag="psB")
                        for k in range(KT):
                            nc.tensor.matmul(
                                psv[:],
                                wvr_sb[:, k, m * P : (m + 1) * P],
                                xt[k][i],
                                start=(k == 0), stop=(k == KT - 1),
                            )
                        # ctx partial: sum_n relu(vr) * exp(relu(qr))
                        scr = deadp.tile([P, NW], F32, name=f"sttscr{s}", tag="sttscr")
                        nc.vector.scalar_tensor_tensor(
                            scr[:], psv[:], 0.0, et[:], Alu.max, Alu.mult,
                            accum_out=ctxp[m][:, i : i + 1],
                        )

                # ---- finalize mask Z and context; layernorm + sigmoid -> s ----
                Zt = smp.tile([P, 1], F32, name=f"Z{s}", tag="Z")
                nc.vector.tensor_reduce(Zt[:], zpart[:], AxX, Alu.add)
                rZ = smp.tile([P, 1], F32, name=f"rZ{s}", tag="rZ")
                nc.vector.reciprocal(rZ[:], Zt[:])
                ctx44 = smp.tile([P, MT], F32, name=f"ctx44{s}", tag="ctx44")
                for m in range(MT):
                    cred = smp.tile([P, 1], F32, name=f"cred{s}_{m}", tag="cred")
                    nc.vector.tensor_reduce(cred[:], ctxp[m][:], AxX, Alu.add)
                    nc.vector.tensor_scalar(
                        ctx44[:, m : m + 1], cred[:], rZ[:], None, Alu.mult
                    )
                lnsum = smp.tile([P, MT], F32, name=f"lnsum{s}", tag="lnsum")
                nc.gpsimd.partition_all_reduce(
                    lnsum[:], ctx44[:], P, bass_isa.ReduceOp.add
                )
                tot = smp.tile([P, 1], F32, name=f"tot{s}", tag="tot")
                nc.vector.tensor_reduce(tot[:], lnsum[:], AxX, Alu.add)
                mu = smp.tile([P, 1], F32, name=f"mu{s}", tag="mu")
                nc.vector.tensor_scalar(mu[:], tot[:], 1.0 / CH, None, Alu.mult)
                d44 = smp.tile([P, MT], F32, name=f"d44{s}", tag="d44")
                nc.vector.tensor_scalar(d44[:], ctx44[:], mu[:], None, Alu.subtract)
                d2 = smp.tile([P, MT], F32, name=f"d2{s}", tag="d2")
                nc.vector.tensor_tensor(d2[:], d44[:], d44[:], Alu.mult)
                vsum = smp.tile([P, MT], F32, name=f"vsum{s}", tag="vsum")
                nc.gpsimd.partition_all_reduce(
                    vsum[:], d2[:], P, bass_isa.ReduceOp.add
                )
                vtot = smp.tile([P, 1], F32, name=f"vtot{s}", tag="vtot")
                nc.vector.tensor_reduce(vtot[:], vsum[:], AxX, Alu.add)
                var = smp.tile([P, 1], F32, name=f"var{s}", tag="var")
                nc.vector.tensor_scalar(var[:], vtot[:], 1.0 / CH, None, Alu.mult)
                # ---- phase B: ql conv in fp8 DoubleRow (2 k-tiles/pass) ----
                # x and w_ql are pre-quantized to e4m3 on the host (w scaled
                # by 64; folded back out in the e44 exp scale). Quantization
                # noise washes out through mean(4096) + near-uniform softmax.
                xq_t = {}
                for i in range(NCH - 1, -1, -1):
                    # reverse order: chunks 0..4 stay resident in the 5-slot
                    # pool for phase C to reuse without re-DMA
                    xq = x8p.tile([P, KT, NW], F8, name=f"xq{s}_{i}", tag="xq8")
                    nc.sync.dma_start(xq[:], xq_d.ap()[s, i])
                    xq_t[i] = xq
                    for m in range(MT):
                        psv = psB.tile([P, NW], F32, name=f"psv{s}b{i}_{m}", tag="psB")
                        for a in range(KT // 2):
                            nc.tensor.matmul(
                                psv[:],
                                wql_sb[:, 2 * a : 2 * a + 2, m * P : (m + 1) * P],
                                xq[:, 2 * a : 2 * a + 2, :],
                                start=(a == 0), stop=(a == KT // 2 - 1),
                                perf_mode=mybir.MatmulPerfMode.DoubleRow,
                            )
                        # relu + accumulate mean partials; alternate engines
                        if m % 2 == 0:
                            scr = psD.tile([P, NW], F32, name=f"qlscr{s}", tag="psD")
                            nc.scalar.activation(
                                scr[:], psv[:], Act.Relu, accum_out=gp[m][:, i : i + 1]
                            )
                        else:
                            scr2 = deadp.tile([P, NW], F32, name=f"sttscr{s}b", tag="sttscr")
                            nc.vector.tensor_scalar(
                                scr2[:], psv[:], 0.0, 0.0, Alu.max, Alu.add,
                                accum_out=gp[m][:, i : i + 1],
                            )
                g44 = smp.tile([P, MT], F32, name=f"g44{s}", tag="g44")
                for m in range(MT):
                    nc.vector.tensor_reduce(g44[:, m : m + 1], gp[m][:], AxX, Alu.add)
                e44 = smp.tile([P, MT], F32, name=f"e44{s}", tag="e44")
                nc.scalar.activation(e44[:], g44[:], Act.Exp, scale=1.0 / (HW * 64.0))
                std = smp.tile([P, 1], F32, name=f"std{s}", tag="std")
                nc.scalar.activation(std[:], var[:], Act.Sqrt, bias=epst[:])
                rstd = smp.tile([P, 1], F32, name=f"rstd{s}", tag="rstd")
                nc.vector.reciprocal(rstd[:], std[:])
                spre = smp.tile([P, MT], F32, name=f"spre{s}", tag="spre")
                nc.vector.tensor_scalar(
                    spre[:], ctx44[:], mu[:], rstd[:], Alu.subtract, Alu.mult
                )
                s44 = smp.tile([P, MT], F32, name=f"s44{s}", tag="s44")
                nc.scalar.activation(s44[:], spre[:], Act.Sigmoid)
                sp44 = smp.tile([P, MT], F32, name=f"sp44{s}", tag="sp44")
                nc.vector.tensor_scalar(sp44[:], s44[:], 1.0, None, Alu.add)

                ze = smp.tile([P, MT], F32, name=f"ze{s}", tag="ze")
                nc.gpsimd.partition_all_reduce(ze[:], e44[:], P, bass_isa.ReduceOp.add)
                zet = smp.tile([P, 1], F32, name=f"zet{s}", tag="zet")
                nc.vector.tensor_reduce(zet[:], ze[:], AxX, Alu.add)
                rZc = smp.tile([P, 1], F32, name=f"rZc{s}", tag="rZc")
                nc.vector.reciprocal(rZc[:], zet[:])
                erep = []
                for m in range(MT):
                    er = erp.tile([P, P], F32R, name=f"erep{s}_{m}", tag="erep")
                    # 1/64 compensates the x64 fp8 scaling of wvl
                    nc.vector.tensor_scalar(
                        er[:], e44[:, m : m + 1].broadcast_to([P, P]),
                        1.0 / 64.0, None, Alu.mult,
                    )
                    erep.append(er)

                # ---- phase C: vl conv -> chan attn -> finale + store ----
                for i in range(NCH):
                    if i + 5 < NCH:
                        # prefetch fp8 chunks evicted by the reverse phase-B
                        xq = x8p.tile([P, KT, NW], F8, name=f"xqc{s}_{i + 5}", tag="xq8")
                        nc.sync.dma_start(xq[:], xq_d.ap()[s, i + 5])
                        xq_t[i + 5] = xq
                    pschan = psA.tile([P, NW], F32, name=f"psc{s}_{i}", tag="psA")
                    thl = []
                    for m in range(MT):
                        psv = psB.tile([P, NW], F32, name=f"psv{s}c{i}_{m}", tag="psB")
                        for a in range(KT // 2):
                            nc.tensor.matmul(
                                psv[:],
                                wvl_sb[:, 2 * a : 2 * a + 2, m * P : (m + 1) * P],
                                xq_t[i][:, 2 * a : 2 * a + 2, :],
                                start=(a == 0), stop=(a == KT // 2 - 1),
                                perf_mode=mybir.MatmulPerfMode.DoubleRow,
                            )
                        th = thp.tile([P, NW], F32R, name=f"th{s}_{i}_{m}", tag="th")
                        nc.scalar.activation(th[:], psv[:], Act.Relu)
                        thl.append(th)
                    # chan partials after all relus so the PE stalls at most on
                    # the last one: rows of pschan are broadcast copies of
                    # sum_c e_g[c] * theta[c, n]
                    for m in range(MT):
                        nc.tensor.matmul(
                            pschan[:], erep[m][:], thl[m][:],
                            start=(m == 0), stop=(m == MT - 1),
                            skip_group_check=True,
                        )
                    chant = actp.tile([P, NW], F32, name=f"ch{s}_{i}", tag="chant", bufs=4)
                    nc.scalar.activation(chant[:], pschan[:], Act.Sigmoid, scale=rZc[:])
                    if s + 1 < S:
                        xc_all.setdefault(s + 1, []).append(emit_x_load(s + 1, i))
                    # finale: seq rows k<4: x*(1 + s*chan); par rows: x*(chan+1+s).
                    # Work is spread across ACT/DVE/GpSimd (~5us per chunk each).
                    for k in range(KT):
                        xf = xt[k][i].bitcast(F32)
                        ot = actp.tile([P, NW], F32, name=f"ot{s}_{i}_{k}", tag="a1", bufs=6)
                        if k < 2:
                            # attn on ACT, mult on DVE
                            nc.scalar.activation(
                                ot[:], chant[:], Act.Copy,
                                scale=s44[:, k : k + 1], bias=1.0,
                            )
                            nc.vector.tensor_tensor(ot[:], ot[:], xf, Alu.mult)
                        elif k < MT:
                            # attn on GpSimd, mult on DVE
                            nc.gpsimd.tensor_scalar(
                                ot[:], chant[:], s44[:, k : k + 1], 1.0,
                                Alu.mult, Alu.add,
                            )
                            nc.vector.tensor_tensor(ot[:], ot[:], xf, Alu.mult)
                        else:
                            # fused attn+mult on DVE
                            nc.vector.scalar_tensor_tensor(
                                ot[:], chant[:], sp44[:, k - MT : k - MT + 1],
                                xf, Alu.add, Alu.mult,
                            )
                        nc.sync.dma_start(
                            out_d.ap()[s, k, :, i * NW : (i + 1) * NW], ot[:]
                        )

    nc.compile()
    return nc


def _prep_inputs(x, w_qr, w_vr, w_ql, w_vl):
    x = np.asarray(x, dtype=np.float32).reshape(B, C, HW)
    wts = {}
    for nm, w in (("wvr", w_vr), ("wql", w_ql), ("wvl", w_vl)):
        w = np.asarray(w, dtype=np.float32)
        # (out, in) -> [P, KT, out]: wts[nm][p, k, o] = w[o, 128k + p]
        wts[nm] = np.ascontiguousarray(w.T.reshape(KT, P, CH).transpose(1, 0, 2))
    q = np.asarray(w_qr, dtype=np.float32).reshape(KT, P).T  # [P, KT]
    wts["wqr"] = np.ascontiguousarray(np.broadcast_to(q[:, :, None], (P, KT, P)))
    # ql runs in fp8e4m3 DoubleRow; scale weights x64 into fp8 range (the
    # 1/64 is folded into the e44 exp scale)
    import ml_dtypes

    f8 = np.dtype(ml_dtypes.float8_e4m3)
    wts["wql"] = (wts["wql"] * 64.0).astype(f8)
    wts["wvl"] = (wts["wvl"] * 64.0).astype(f8)
    in_maps = []
    for c in range(N_CORES):
        m = dict(wts)
        # [S, chunk, P, KT, NW]: m["x"][s, i, p, k, n] = x[s, 128k+p, 512i+n]
        m["x"] = np.ascontiguousarray(
            x[S * c : S * (c + 1)]
            .reshape(S, KT, P, NCH, NW)
            .transpose(0, 3, 2, 1, 4)
        )
        m["xq"] = m["x"].astype(f8)
        in_maps.append(m)
    return in_maps


def _run(x, w_qr, w_vr, w_ql, w_vl, trace=False):
    if "nc" not in _cache:
        _cache["nc"] = _build()
    nc = _cache["nc"]
    in_maps = _prep_inputs(x, w_qr, w_vr, w_ql, w_vl)
    res = bass_utils.run_bass_kernel_spmd(
        nc, in_maps, core_ids=list(range(N_CORES)), trace=trace
    )
    out = np.empty((B, C, HW), np.float32)
    for c in range(N_CORES):
        out[S * c : S * (c + 1)] = res.results[c]["out"].reshape(S, C, HW)
    return out.reshape(B, C, H, W), res


def kernel(x, w_qr, w_vr, w_ql, w_vl):
    out, _ = _run(x, w_qr, w_vr, w_ql, w_vl, trace=False)
    return out
